# revision 3
# baseline (speedup 1.0000x reference)
"""Trainium2 Bass kernel for nn_MaxPoolingMatching.

    Y[l,m,p] = sum_d s1[l,d] * k2[p,d] * s2[m,d]          (k2 = k*k)
    out[l,p] = rinv1[l,p] * max_m ( Y[l,m,p] * rinv2[m,p] )

All rhsKn-production ops use fully contiguous [128, 512/1024] APs so DVE
2x/4x perf modes engage.  Assignment (measured rates):
  - W1 (x k2): slab0 on DVE tensor_scalar, slabs 1-4 on ScalarE
  - W2 (x rinv2): slabs 0-1 on DVE TT (bf16 2x), slabs 2-4 on GpSimd,
    both as p-pair [128, 1024] instructions, in-place
  - W3 (max over m): batched DVE tensor_reduce from PSUM
  - main GEMM bf16 N=512 (p-pairs via strided moving AP), dc-outer shared
    stationary, slab-outer loop so consumption chases production
"""

import sys

import numpy as np

if "/opt/trn_rl_repo" not in sys.path:
    sys.path.insert(0, "/opt/trn_rl_repo")

B, L, D, P = 16, 256, 256, 20
NCORES = 8
BLOC = B // NCORES
LC = L // 128
DC = D // 128
PSLAB = 4
N_SLABS = P // PSLAB
W1_DVE_SLABS = 1   # leading slabs with W1 on DVE tensor_scalar
W2_DVE_SLABS = 2   # leading slabs with W2 on DVE TT (rest GpSimd)

_NC_CACHE = {}


def build_nc():
    import concourse.bass as bass
    import concourse.bacc as bacc
    import concourse.tile as tile
    from concourse import mybir
    from concourse.masks import make_identity
    from contextlib import ExitStack

    f32 = mybir.dt.float32
    bf16 = mybir.dt.bfloat16
    Alu = mybir.AluOpType
    Act = mybir.ActivationFunctionType
    X = mybir.AxisListType.X

    nc = bacc.Bacc("TRN2", target_bir_lowering=False, debug=False)
    s1_d = nc.dram_tensor("sent1", [BLOC, L, D], f32, kind="ExternalInput").ap()
    s2_d = nc.dram_tensor("sent2", [BLOC, L, D], f32, kind="ExternalInput").ap()
    kr_d = nc.dram_tensor("kernel", [P, D], f32, kind="ExternalInput").ap()
    out_d = nc.dram_tensor("out", [BLOC, L, P], f32, kind="ExternalOutput").ap()

    with ExitStack() as ctx:
        tc = ctx.enter_context(tile.TileContext(nc))
        consts = ctx.enter_context(tc.tile_pool(name="consts", bufs=1))
        nat = ctx.enter_context(tc.tile_pool(name="nat", bufs=4))
        big = ctx.enter_context(tc.tile_pool(name="big", bufs=1))
        small = ctx.enter_context(tc.tile_pool(name="small", bufs=6))
        outp = ctx.enter_context(tc.tile_pool(name="outp", bufs=2))
        dramp = ctx.enter_context(tc.tile_pool(name="dram", bufs=2, space="DRAM"))
        ps_z = ctx.enter_context(tc.tile_pool(name="ps_z", bufs=4, space="PSUM"))

        ident = consts.tile([128, 128], f32, tag="ident")
        make_identity(nc, ident)
        eps_t = consts.tile([128, 1], f32, tag="eps")
        nc.vector.memset(eps_t, 1e-12)

        def psum_tile():
            return ps_z.tile([128, PSLAB, L], f32, tag="z", name="psz")

        # ---- k2 = kernel^2 -> k2T fp32 ------------------------------------
        kr = consts.tile([P, D], f32, tag="kr")
        nc.sync.dma_start(out=kr, in_=kr_d)
        k2 = consts.tile([P, D], f32, tag="k2")
        nc.gpsimd.tensor_mul(k2, kr, kr)
        k2T = consts.tile([128, DC, P], f32, tag="k2T")
        pk = psum_tile()
        for dc in range(DC):
            nc.tensor.transpose(
                pk[:, dc, :P], k2[:, dc * 128 : (dc + 1) * 128], ident[:P, :P]
            )
        nc.scalar.copy(out=k2T[:, :, :], in_=pk[:, :DC, :P])

        # ---- load + PE-transpose inputs (one [128,256] copy-out per tile) -
        s1T = big.tile([128, DC, BLOC, L], bf16, tag="s1T")
        s2T = big.tile([128, DC, BLOC, L], bf16, tag="s2T")

        def load_T(src_d, dstT, b, lc):
            natt = nat.tile([128, D], f32, tag="nat")
            nc.sync.dma_start(out=natt, in_=src_d[b, lc * 128 : (lc + 1) * 128, :])
            pst = psum_tile()
            for dc in range(DC):
                nc.tensor.transpose(
                    pst[:, dc, :128], natt[:, dc * 128 : (dc + 1) * 128], ident
                )
            # single strided copy-out for both d-chunks, casting to bf16
            nc.scalar.copy(
                out=dstT[:, :, b, lc * 128 : (lc + 1) * 128],
                in_=pst[:, :DC, :128],
            )

        for b in range(BLOC):
            for lc in range(LC):
                load_T(s2_d, s2T, b, lc)
        for b in range(BLOC):
            for lc in range(LC):
                load_T(s1_d, s1T, b, lc)

        # squares (GpSimd, bf16 in -> f32 out) for fp32 norm matmuls
        s2Tsq = big.tile([128, DC, BLOC, L], f32, tag="s2Tsq")
        for b in range(BLOC):
            nc.gpsimd.tensor_mul(s2Tsq[:, :, b, :], s2T[:, :, b, :], s2T[:, :, b, :])

        # ---- rinv2 per batch -> bf16 broadcast r2bb [128, P, BLOC, L] -----
        r2bb = big.tile([128, P, BLOC, L], bf16, tag="r2bb")
        for b in range(BLOC):
            psn2 = psum_tile()
            for dc in range(DC):
                nc.tensor.matmul(
                    psn2[:P, 0, :],
                    k2T[:, dc, :],
                    s2Tsq[:, dc, b, :],
                    start=(dc == 0),
                    stop=(dc == DC - 1),
                )
            sq2 = small.tile([P, L], f32, tag="sq2")
            nc.scalar.activation(
                out=sq2, in_=psn2[:P, 0, :], func=Act.Sqrt, bias=eps_t[:P], scale=1.0
            )
            r2 = small.tile([P, L], f32, tag="r2")
            nc.vector.reciprocal_approx_fast(out=r2, in_=sq2)
            r2b16 = small.tile([P, L], bf16, tag="r2b16")
            nc.vector.tensor_copy(out=r2b16, in_=r2)
            r2d = dramp.tile([P, L], bf16, tag="r2d")
            nc.sync.dma_start(out=r2d, in_=r2b16)
            r2d_flat = r2d.rearrange("a b -> (a b)")
            for ps in range(0, P, PSLAB):
                chunk = r2d_flat[ps * L : (ps + PSLAB) * L]
                chunk_bcast = bass.AP(
                    tensor=chunk.tensor,
                    offset=chunk.offset,
                    ap=[[0, 128]] + list(chunk.ap),
                )
                nc.sync.dma_start(
                    out=r2bb[:, ps : ps + PSLAB, b, :], in_=chunk_bcast
                )

        # s1 squares + rinv1 (late)
        s1Tsq = big.tile([128, DC, BLOC, L], f32, tag="s1Tsq")
        for b in range(BLOC):
            nc.gpsimd.tensor_mul(s1Tsq[:, :, b, :], s1T[:, :, b, :], s1T[:, :, b, :])
        rinv1 = {}
        for b in range(BLOC):
            for lc in range(LC):
                psn = psum_tile()
                for dc in range(DC):
                    nc.tensor.matmul(
                        psn[:, 0, :P],
                        s1Tsq[:, dc, b, lc * 128 : (lc + 1) * 128],
                        k2T[:, dc, :],
                        start=(dc == 0),
                        stop=(dc == DC - 1),
                    )
                sq1 = small.tile([128, P], f32, tag="sq1")
                nc.scalar.activation(
                    out=sq1, in_=psn[:, 0, :P], func=Act.Sqrt, bias=eps_t, scale=1.0
                )
                r1 = small.tile([128, P], f32, tag="r1")
                nc.vector.reciprocal_approx_fast(out=r1, in_=sq1)
                rinv1[b, lc] = r1

        # ---- rhsKn [128, DC, P, BLOC, L] bf16 (fully contiguous units) ----
        rhsKn = big.tile([128, DC, P, BLOC, L], bf16, tag="rhsKn")

        for si in range(N_SLABS):
            ps = si * PSLAB
            # W1: x k2 (per-partition scalar)
            for p in range(ps, ps + PSLAB):
                for dc in range(DC):
                    o = rhsKn[:, dc, p, :, :]
                    i0 = s2T[:, dc, :, :]
                    sc = k2T[:, dc, p : p + 1]
                    if si < W1_DVE_SLABS:
                        nc.vector.tensor_scalar_mul(o, i0, sc)
                    else:
                        nc.scalar.activation(out=o, in_=i0, func=Act.Copy, scale=sc)
            # W2: x rinv2, in-place, p-pair [128, 1024] contiguous
            for p in range(ps, ps + PSLAB, 2):
                for dc in range(DC):
                    o = rhsKn[:, dc, p : p + 2, :, :]
                    i1 = r2bb[:, p : p + 2, :, :]
                    eng = nc.vector if si < W2_DVE_SLABS else nc.gpsimd
                    eng.tensor_mul(o, o, i1)

        # ---- main loop: slab-outer, dc-outer, bf16 pair matmuls -----------
        maxt = {}
        for b in range(BLOC):
            for lc in range(LC):
                maxt[b, lc] = outp.tile(
                    [128, P], f32, tag=f"maxt{b}{lc}", name=f"maxt{b}{lc}"
                )
        for ps in range(0, P, PSLAB):
            for b in range(BLOC):
                for lc in range(LC):
                    psz = psum_tile()
                    for dc in range(DC):
                        for i in range(0, PSLAB, 2):
                            # moving operand: p-pair, strided [2, 256] AP
                            nc.tensor.matmul(
                                psz[:, i : i + 2, :],
                                s1T[:, dc, b, lc * 128 : (lc + 1) * 128],
                                rhsKn[:, dc, ps + i : ps + i + 2, b, :],
                                start=(dc == 0),
                                stop=(dc == DC - 1),
                                skip_group_check=True,
                            )
                    nc.vector.tensor_reduce(
                        out=maxt[b, lc][:, ps : ps + PSLAB],
                        in_=psz,
                        axis=X,
                        op=Alu.max,
                    )
        for b in range(BLOC):
            for lc in range(LC):
                outt = outp.tile([128, P], f32, tag="outt")
                nc.vector.tensor_mul(outt, maxt[b, lc], rinv1[b, lc])
                nc.sync.dma_start(
                    out=out_d[b, lc * 128 : (lc + 1) * 128, :], in_=outt
                )

    nc.compile()
    return nc


def _get_nc():
    if "nc" not in _NC_CACHE:
        _NC_CACHE["nc"] = build_nc()
    return _NC_CACHE["nc"]


def run(inputs, trace=False, trace_kwargs=None):
    from concourse.bass_utils import run_bass_kernel_spmd

    nc = _get_nc()
    sent1 = np.ascontiguousarray(np.asarray(inputs["sent1"], dtype=np.float32))
    sent2 = np.ascontiguousarray(np.asarray(inputs["sent2"], dtype=np.float32))
    kr = np.ascontiguousarray(np.asarray(inputs["kernel"], dtype=np.float32))
    in_maps = [
        {
            "sent1": sent1[i * BLOC : (i + 1) * BLOC],
            "sent2": sent2[i * BLOC : (i + 1) * BLOC],
            "kernel": kr,
        }
        for i in range(NCORES)
    ]
    res = run_bass_kernel_spmd(
        nc,
        in_maps,
        core_ids=list(range(NCORES)),
        trace=trace,
        **(trace_kwargs or {}),
    )
    out = np.concatenate([res.results[i]["out"] for i in range(NCORES)], axis=0)
    return out, res


def kernel(sent1, sent2, kernel):
    out, _ = run({"sent1": sent1, "sent2": sent2, "kernel": kernel})
    return out
